# revision 8
# baseline (speedup 1.0000x reference)
"""BaseLayer MoE gate (balanced assignment) for Trainium2, 8 NeuronCores.

Strategy:
  - The roofline-dominant work is the token->expert affinity matmul
    X[16384, 2048] @ C.T[2048, 16] (reads 134 MB).  Tokens are sharded
    8 ways; each core computes aff.T[16, 2048] for its 2048-token shard
    via PSUM-accumulated PE matmuls (contraction over d_model in 16
    chunks of 128).
  - The auction-based balanced assignment operates on the tiny
    [16, 16384] affinity matrix and is an inherently sequential,
    data-dependent while loop (converges in ~11 iterations here); it
    runs on host as an exact replica of the reference semantics.
"""

import numpy as np

D = 2048
E = 16
N_CORES = 8
TOK_PER_CORE = 2048
N_TOK = N_CORES * TOK_PER_CORE
TOK_BLK = 512
N_BLK = TOK_PER_CORE // TOK_BLK  # 4
K_CHUNKS = D // 128  # 16

_cache = {}


def _build_nc(mm_dtype_name="float32"):
    import concourse.tile as tile
    from concourse import bacc, mybir

    f32 = mybir.dt.float32
    mm_dt = getattr(mybir.dt, mm_dtype_name)

    nc = bacc.Bacc(
        "TRN2", target_bir_lowering=False, debug=False, num_devices=N_CORES
    )
    xt = nc.declare_dram_parameter("xt", [D, TOK_PER_CORE], f32, isOutput=False)
    ct = nc.declare_dram_parameter("ct", [D, E], f32, isOutput=False)
    aff = nc.declare_dram_parameter("aff", [TOK_PER_CORE, E], f32, isOutput=True)

    HALF = TOK_PER_CORE // 2  # 1024 tokens per half (8 PSUM banks)
    N_TILE = HALF // 128      # 8 token tiles of 128 per half

    with tile.TileContext(nc) as tc:
        with tc.tile_pool(name="cpool", bufs=1) as cpool, \
             tc.tile_pool(name="xpool", bufs=4) as xpool, \
             tc.tile_pool(name="opool", bufs=4) as opool, \
             tc.tile_pool(name="psum", bufs=8, space="PSUM") as psum_pool:
            ct_sb = cpool.tile([128, K_CHUNKS, E], f32)
            nc.sync.dma_start(
                out=ct_sb[:], in_=ct[:].rearrange("(k p) e -> p k e", p=128)
            )
            for h in range(2):
                ps = [
                    psum_pool.tile([128, E], f32, tag="ps", name=f"ps_{h}_{i}")
                    for i in range(N_TILE)
                ]
                for k in range(K_CHUNKS):
                    xk = xpool.tile([128, HALF], f32)
                    nc.sync.dma_start(
                        out=xk[:],
                        in_=xt[k * 128:(k + 1) * 128, h * HALF:(h + 1) * HALF],
                    )
                    for i in range(N_TILE):
                        nc.tensor.matmul(
                            ps[i][:],
                            xk[:, i * 128:(i + 1) * 128].bitcast(mm_dt),
                            ct_sb[:, k, :].bitcast(mm_dt),
                            start=(k == 0), stop=(k == K_CHUNKS - 1),
                        )
                for i in range(N_TILE):
                    ob = opool.tile([128, E], f32)
                    nc.scalar.copy(ob[:], ps[i][:])
                    t0 = (h * N_TILE + i) * 128
                    nc.sync.dma_start(out=aff[t0:t0 + 128, :], in_=ob[:])
    nc.compile()
    return nc


def _get_nc():
    if "nc" not in _cache:
        _cache["nc"] = _build_nc()
    return _cache["nc"]


def _device_affinities_T(x_flat, centroids):
    """Run the 8-core bass kernel; return aff.T [E, N_TOK] float32."""
    from concourse.bass_utils import run_bass_kernel_spmd

    ct = np.ascontiguousarray(centroids.T).astype(np.float32, copy=False)
    in_maps = []
    for i in range(N_CORES):
        shard = x_flat[i * TOK_PER_CORE:(i + 1) * TOK_PER_CORE]
        in_maps.append(
            {"xt": np.ascontiguousarray(shard.T), "ct": ct}
        )
    nc = _get_nc()
    res = run_bass_kernel_spmd(nc, in_maps, list(range(N_CORES)))
    aff_full = np.concatenate(
        [res.results[i]["aff"] for i in range(N_CORES)], axis=0
    )  # [N_TOK, E]
    return np.ascontiguousarray(aff_full.T)


def _balanced_assignment_host(s):
    """Exact host replica of the reference auction on s = scores.T [E, N]."""
    ok = np.isfinite(s)
    if not ok.all():
        fmin = np.min(np.where(ok, s, np.inf))
        s = np.where(ok, s, fmin).astype(np.float32)
    eps = np.maximum(
        np.float32((np.float32(s.max()) - np.float32(s.min())) / np.float32(50.0)),
        np.float32(1e-4),
    )
    E_, N = s.shape
    jpw = N // E_
    rows = np.arange(E_)[:, None]
    jobs_idx = np.arange(N)
    MAX_GREEDY = 100
    HARD_CAP = 200

    value = s.copy()
    cost = np.zeros(N, np.float32)
    prev_bidders = np.zeros(N, np.int32)
    prev_have = np.zeros(N, bool)
    it = 0
    top_index = None
    while it < HARD_CAP:
        order = np.argsort(-value, axis=1, kind="stable")
        top_index = order[:, : jpw + 1]
        top_values = np.take_along_axis(value, top_index, axis=1)
        bid_incr = top_values[:, :jpw] - top_values[:, jpw:] + eps
        bids = np.zeros_like(s)
        bids[rows, top_index[:, :jpw]] = bid_incr
        bids[prev_bidders, jobs_idx] = np.where(
            prev_have, eps, bids[prev_bidders, jobs_idx]
        )
        high_bids = bids.max(axis=0)
        high_bidders = bids.argmax(axis=0).astype(np.int32)
        have_bids = high_bids > 0
        done = bool(np.all(have_bids))
        cost = (cost + high_bids).astype(np.float32)
        value = (s - cost).astype(np.float32)
        if it < MAX_GREEDY:
            upd = np.full(N, np.inf, np.float32)
        else:
            upd = s[high_bidders, jobs_idx]
        value[high_bidders, jobs_idx] = np.where(
            have_bids, upd, value[high_bidders, jobs_idx]
        )
        prev_bidders = high_bidders
        prev_have = have_bids
        it += 1
        if done:
            break
    return top_index[:, :jpw].astype(np.int32)


def kernel(input_features, expert_centroids):
    x_flat = np.ascontiguousarray(
        input_features.reshape(-1, input_features.shape[-1])
    ).astype(np.float32, copy=False)
    afft = _device_affinities_T(x_flat, expert_centroids)  # [E, N]
    top_idx = _balanced_assignment_host(afft)
    top_value = np.take_along_axis(afft, top_idx, axis=1).astype(np.float32)
    return top_idx, top_value


# revision 38
# speedup vs baseline: 2.0664x; 2.0664x over previous
"""BaseLayer MoE gate (balanced assignment) for Trainium2, 8 NeuronCores.

Strategy:
  - The roofline-dominant work is the token->expert affinity matmul
    X[16384, 2048] @ C.T[2048, 16] (reads 134 MB; the kernel is
    HBM-bandwidth bound).  Tokens are sharded 8 ways; each core computes
    aff.T[16, 2048] for its 2048-token shard.
  - Per core: X-shard is fed pre-transposed ([d_model, tok], so the
    d_model contraction lands on SBUF partitions) and streamed as eight
    2MB fused DMA loads; the fp32 matmul uses PE *column tiling*
    (tile_position=(0, 32b)) to run the four 512-token blocks
    concurrently in the four 32-column PE quadrants (fp32 moving costs
    4 cycles/row, so without packing the PE would be the bottleneck).
    Contraction accumulates over 16 k-chunks into one PSUM bank;
    evacuation alternates scalar/vector copies and both HWDGE queues.
  - fp32 precision end-to-end is required: the auction's final
    assignment is stable under affinity perturbations up to ~1e-6 but
    flips thousands of indices by 1e-5, which rules out bf16/fp32r
    tricks (verified empirically).
  - The auction-based balanced assignment operates on the tiny
    [16, 16384] affinity matrix and is an inherently sequential,
    data-dependent while loop (converges in ~11 iterations here); it
    runs on host as an exact bit-level replica of the reference
    semantics (verified to reproduce jax.lax.top_k tie-breaking and the
    full reference trajectory).
"""

import numpy as np

D = 2048
E = 16
N_CORES = 8
TOK_PER_CORE = 2048
N_TOK = N_CORES * TOK_PER_CORE
TOK_BLK = 512
N_BLK = TOK_PER_CORE // TOK_BLK  # 4
K_CHUNKS = D // 128  # 16

_cache = {}


def _build_nc(mm_dtype_name="float32"):
    import concourse.tile as tile
    from concourse import bacc, mybir

    f32 = mybir.dt.float32
    mm_dt = getattr(mybir.dt, mm_dtype_name)

    nc = bacc.Bacc(
        "TRN2", target_bir_lowering=False, debug=False, num_devices=N_CORES
    )
    xt = nc.declare_dram_parameter("xt", [D, TOK_PER_CORE], f32, isOutput=False)
    # ctp: centroids pre-arranged on host as [128, K_CHUNKS, E]
    ctp = nc.declare_dram_parameter("ctp", [128, K_CHUNKS, E], f32, isOutput=False)
    afft = nc.declare_dram_parameter("afft", [E, TOK_PER_CORE], f32, isOutput=True)

    with tile.TileContext(nc) as tc:
        with tc.tile_pool(name="cpool", bufs=1) as cpool, \
             tc.tile_pool(name="xpool", bufs=7) as xpool, \
             tc.tile_pool(name="opool", bufs=4) as opool, \
             tc.tile_pool(name="psum", bufs=1, space="PSUM") as psum_pool:
            ct_sb = cpool.tile([128, K_CHUNKS, E], f32)
            nc.scalar.dma_start(out=ct_sb[:], in_=ctp[:])
            # One PSUM bank [128, TOK_BLK]; col tile b owns partitions
            # 32b..32b+E (M=16 rows of its 32-partition quadrant).
            ps = psum_pool.tile([128, TOK_BLK], f32)
            for kp in range(K_CHUNKS // 2):
                # fused k-pair load: [128, 2, TOK_PER_CORE] (2MB) in one dma
                xk = xpool.tile(
                    [128, 2, TOK_PER_CORE], f32, tag="xk", name=f"xk_{kp}"
                )
                if kp == 0:
                    # first pair: one chunk on each HWDGE engine, in parallel,
                    # so compute can start as early as possible
                    nc.scalar.dma_start(out=xk[:, 0, :], in_=xt[0:128, :])
                    nc.sync.dma_start(out=xk[:, 1, :], in_=xt[128:256, :])
                else:
                    src = xt[
                        2 * kp * 128:(2 * kp + 2) * 128, :
                    ].rearrange("(kk p) t -> p kk t", kk=2)
                    nc.sync.dma_start(out=xk[:], in_=src)
                for kk in range(2):
                    k = 2 * kp + kk
                    for b in range(N_BLK):
                        nc.tensor.matmul(
                            ps[32 * b:32 * b + E, :],
                            ct_sb[:, k, :].bitcast(mm_dt),
                            xk[:, kk, b * TOK_BLK:(b + 1) * TOK_BLK].bitcast(mm_dt),
                            start=(k == 0), stop=(k == K_CHUNKS - 1),
                            tile_position=(0, 32 * b),
                        )
            for b in range(N_BLK):
                ob = opool.tile([E, TOK_BLK], f32, tag="ob", name=f"ob_{b}")
                if b % 2 == 0:
                    nc.scalar.copy(ob[:], ps[32 * b:32 * b + E, :])
                else:
                    nc.vector.tensor_copy(ob[:], ps[32 * b:32 * b + E, :])
                deng = nc.sync if b < 2 else nc.scalar
                deng.dma_start(
                    out=afft[:, b * TOK_BLK:(b + 1) * TOK_BLK], in_=ob[:]
                )
    nc.compile()
    return nc


def _get_nc():
    if "nc" not in _cache:
        _cache["nc"] = _build_nc()
    return _cache["nc"]


def _make_in_maps(x_flat, centroids):
    # [E, D] -> C.T [D, E] -> [K_CHUNKS, 128, E] -> [128, K_CHUNKS, E]
    ctp = np.ascontiguousarray(
        centroids.T.astype(np.float32, copy=False)
        .reshape(K_CHUNKS, 128, E)
        .transpose(1, 0, 2)
    )
    in_maps = []
    for i in range(N_CORES):
        shard = x_flat[i * TOK_PER_CORE:(i + 1) * TOK_PER_CORE]
        in_maps.append(
            {"xt": np.ascontiguousarray(shard.T), "ctp": ctp}
        )
    return in_maps


def _device_affinities_T(x_flat, centroids):
    """Run the 8-core bass kernel; return aff.T [E, N_TOK] float32."""
    from concourse.bass_utils import run_bass_kernel_spmd

    in_maps = _make_in_maps(x_flat, centroids)
    nc = _get_nc()
    res = run_bass_kernel_spmd(nc, in_maps, list(range(N_CORES)))
    return np.concatenate(
        [res.results[i]["afft"] for i in range(N_CORES)], axis=1
    )  # [E, N_TOK]


def _balanced_assignment_host(s):
    """Exact host replica of the reference auction on s = scores.T [E, N]."""
    ok = np.isfinite(s)
    if not ok.all():
        fmin = np.min(np.where(ok, s, np.inf))
        s = np.where(ok, s, fmin).astype(np.float32)
    eps = np.maximum(
        np.float32((np.float32(s.max()) - np.float32(s.min())) / np.float32(50.0)),
        np.float32(1e-4),
    )
    E_, N = s.shape
    jpw = N // E_
    rows = np.arange(E_)[:, None]
    jobs_idx = np.arange(N)
    MAX_GREEDY = 100
    HARD_CAP = 200

    value = s.copy()
    cost = np.zeros(N, np.float32)
    prev_bidders = np.zeros(N, np.int32)
    prev_have = np.zeros(N, bool)
    it = 0
    top_index = None
    while it < HARD_CAP:
        order = np.argsort(-value, axis=1, kind="stable")
        top_index = order[:, : jpw + 1]
        top_values = np.take_along_axis(value, top_index, axis=1)
        bid_incr = top_values[:, :jpw] - top_values[:, jpw:] + eps
        bids = np.zeros_like(s)
        bids[rows, top_index[:, :jpw]] = bid_incr
        bids[prev_bidders, jobs_idx] = np.where(
            prev_have, eps, bids[prev_bidders, jobs_idx]
        )
        high_bids = bids.max(axis=0)
        high_bidders = bids.argmax(axis=0).astype(np.int32)
        have_bids = high_bids > 0
        done = bool(np.all(have_bids))
        cost = (cost + high_bids).astype(np.float32)
        value = (s - cost).astype(np.float32)
        if it < MAX_GREEDY:
            upd = np.full(N, np.inf, np.float32)
        else:
            upd = s[high_bidders, jobs_idx]
        value[high_bidders, jobs_idx] = np.where(
            have_bids, upd, value[high_bidders, jobs_idx]
        )
        prev_bidders = high_bidders
        prev_have = have_bids
        it += 1
        if done:
            break
    return top_index[:, :jpw].astype(np.int32)


def kernel(input_features, expert_centroids):
    x_flat = np.ascontiguousarray(
        input_features.reshape(-1, input_features.shape[-1])
    ).astype(np.float32, copy=False)
    afft = _device_affinities_T(x_flat, expert_centroids)  # [E, N]
    top_idx = _balanced_assignment_host(afft)
    top_value = np.take_along_axis(afft, top_idx, axis=1).astype(np.float32)
    return top_idx, top_value


# revision 39
# speedup vs baseline: 2.1294x; 1.0305x over previous
"""BaseLayer MoE gate (balanced assignment) for Trainium2, 8 NeuronCores.

Strategy:
  - The roofline-dominant work is the token->expert affinity matmul
    X[16384, 2048] @ C.T[2048, 16] (reads 134 MB; the kernel is
    HBM-bandwidth bound).  Tokens are sharded 8 ways; each core computes
    aff.T[16, 2048] for its 2048-token shard.
  - Per core: X-shard is fed pre-transposed ([d_model, tok], so the
    d_model contraction lands on SBUF partitions) and streamed as eight
    2MB fused DMA loads; the fp32 matmul uses PE *column tiling*
    (tile_position=(0, 32b)) to run the four 512-token blocks
    concurrently in the four 32-column PE quadrants (fp32 moving costs
    4 cycles/row, so without packing the PE would be the bottleneck).
    Contraction accumulates over 16 k-chunks into one PSUM bank;
    evacuation alternates scalar/vector copies and both HWDGE queues.
  - fp32 precision end-to-end is required: the auction's final
    assignment is stable under affinity perturbations up to ~1e-6 but
    flips thousands of indices by 1e-5, which rules out bf16/fp32r
    tricks (verified empirically).
  - The auction-based balanced assignment operates on the tiny
    [16, 16384] affinity matrix and is an inherently sequential,
    data-dependent while loop (converges in ~11 iterations here); it
    runs on host as an exact bit-level replica of the reference
    semantics (verified to reproduce jax.lax.top_k tie-breaking and the
    full reference trajectory).
"""

import numpy as np

D = 2048
E = 16
N_CORES = 8
TOK_PER_CORE = 2048
N_TOK = N_CORES * TOK_PER_CORE
TOK_BLK = 512
N_BLK = TOK_PER_CORE // TOK_BLK  # 4
K_CHUNKS = D // 128  # 16

_cache = {}


def _build_nc(mm_dtype_name="float32"):
    import concourse.tile as tile
    from concourse import bacc, mybir

    f32 = mybir.dt.float32
    mm_dt = getattr(mybir.dt, mm_dtype_name)

    nc = bacc.Bacc(
        "TRN2", target_bir_lowering=False, debug=False, num_devices=N_CORES
    )
    xt = nc.declare_dram_parameter("xt", [D, TOK_PER_CORE], f32, isOutput=False)
    # ctp: centroids pre-arranged on host as [128, K_CHUNKS, E]
    ctp = nc.declare_dram_parameter("ctp", [128, K_CHUNKS, E], f32, isOutput=False)
    afft = nc.declare_dram_parameter("afft", [E, TOK_PER_CORE], f32, isOutput=True)

    with tile.TileContext(nc) as tc:
        with tc.tile_pool(name="cpool", bufs=1) as cpool, \
             tc.tile_pool(name="xpool", bufs=7) as xpool, \
             tc.tile_pool(name="opool", bufs=4) as opool, \
             tc.tile_pool(name="psum", bufs=1, space="PSUM") as psum_pool:
            ct_sb = cpool.tile([128, K_CHUNKS, E], f32)
            nc.scalar.dma_start(out=ct_sb[:], in_=ctp[:])
            # One PSUM bank [128, TOK_BLK]; col tile b owns partitions
            # 32b..32b+E (M=16 rows of its 32-partition quadrant).
            ps = psum_pool.tile([128, TOK_BLK], f32)
            for kp in range(K_CHUNKS // 2):
                # fused k-pair load: [128, 2, TOK_PER_CORE] (2MB) in one dma
                xk = xpool.tile(
                    [128, 2, TOK_PER_CORE], f32, tag="xk", name=f"xk_{kp}"
                )
                if kp == 0:
                    # first pair: one chunk on each HWDGE engine, in parallel,
                    # so compute can start as early as possible
                    nc.scalar.dma_start(out=xk[:, 0, :], in_=xt[0:128, :])
                    nc.sync.dma_start(out=xk[:, 1, :], in_=xt[128:256, :])
                else:
                    src = xt[
                        2 * kp * 128:(2 * kp + 2) * 128, :
                    ].rearrange("(kk p) t -> p kk t", kk=2)
                    nc.sync.dma_start(out=xk[:], in_=src)
                for kk in range(2):
                    k = 2 * kp + kk
                    for b in range(N_BLK):
                        nc.tensor.matmul(
                            ps[32 * b:32 * b + E, :],
                            ct_sb[:, k, :].bitcast(mm_dt),
                            xk[:, kk, b * TOK_BLK:(b + 1) * TOK_BLK].bitcast(mm_dt),
                            start=(k == 0), stop=(k == K_CHUNKS - 1),
                            tile_position=(0, 32 * b),
                        )
            for b in range(N_BLK):
                ob = opool.tile([E, TOK_BLK], f32, tag="ob", name=f"ob_{b}")
                if b % 2 == 0:
                    nc.scalar.copy(ob[:], ps[32 * b:32 * b + E, :])
                else:
                    nc.vector.tensor_copy(ob[:], ps[32 * b:32 * b + E, :])
                deng = nc.sync if b < 2 else nc.scalar
                deng.dma_start(
                    out=afft[:, b * TOK_BLK:(b + 1) * TOK_BLK], in_=ob[:]
                )
    nc.compile()
    return nc


def _get_nc():
    if "nc" not in _cache:
        _cache["nc"] = _build_nc()
    return _cache["nc"]


def _make_in_maps(x_flat, centroids):
    # [E, D] -> C.T [D, E] -> [K_CHUNKS, 128, E] -> [128, K_CHUNKS, E]
    ctp = np.ascontiguousarray(
        centroids.T.astype(np.float32, copy=False)
        .reshape(K_CHUNKS, 128, E)
        .transpose(1, 0, 2)
    )
    in_maps = []
    for i in range(N_CORES):
        shard = x_flat[i * TOK_PER_CORE:(i + 1) * TOK_PER_CORE]
        in_maps.append(
            {"xt": np.ascontiguousarray(shard.T), "ctp": ctp}
        )
    return in_maps


def _axon_available():
    """True if this process's jax can see the 8 NeuronCores."""
    try:
        import jax

        return len(jax.devices()) >= N_CORES and jax.default_backend() != "cpu"
    except Exception:
        return False


def _device_affinities_T(x_flat, centroids):
    """Run the 8-core bass kernel; return aff.T [E, N_TOK] float32."""
    if not _axon_available():
        return _device_affinities_T_subprocess(x_flat, centroids)
    from concourse.bass_utils import run_bass_kernel_spmd

    in_maps = _make_in_maps(x_flat, centroids)
    nc = _get_nc()
    res = run_bass_kernel_spmd(nc, in_maps, list(range(N_CORES)))
    return np.concatenate(
        [res.results[i]["afft"] for i in range(N_CORES)], axis=1
    )  # [E, N_TOK]


def _device_affinities_T_subprocess(x_flat, centroids):
    """Fallback when the calling process pinned jax to CPU: run the device
    kernel in a child process where the neuron/axon PJRT plugin can boot."""
    import os
    import subprocess
    import sys
    import tempfile

    here = os.path.dirname(os.path.abspath(__file__))
    with tempfile.TemporaryDirectory() as td:
        np.save(os.path.join(td, "x.npy"), x_flat)
        np.save(os.path.join(td, "c.npy"), centroids)
        prog = (
            "import sys, numpy as np\n"
            f"sys.path.insert(0, {here!r})\n"
            "import kernel as _k\n"
            f"x = np.load({os.path.join(td, 'x.npy')!r})\n"
            f"c = np.load({os.path.join(td, 'c.npy')!r})\n"
            "a = _k._device_affinities_T(x, c)\n"
            f"np.save({os.path.join(td, 'a.npy')!r}, a)\n"
        )
        env = dict(os.environ)
        env.pop("JAX_PLATFORMS", None)
        env["JAX_PLATFORMS"] = "axon"
        subprocess.run(
            [sys.executable, "-c", prog], env=env, check=True,
            stdout=subprocess.DEVNULL, stderr=subprocess.DEVNULL,
        )
        return np.load(os.path.join(td, "a.npy"))


def _balanced_assignment_host(s):
    """Exact host replica of the reference auction on s = scores.T [E, N]."""
    ok = np.isfinite(s)
    if not ok.all():
        fmin = np.min(np.where(ok, s, np.inf))
        s = np.where(ok, s, fmin).astype(np.float32)
    eps = np.maximum(
        np.float32((np.float32(s.max()) - np.float32(s.min())) / np.float32(50.0)),
        np.float32(1e-4),
    )
    E_, N = s.shape
    jpw = N // E_
    rows = np.arange(E_)[:, None]
    jobs_idx = np.arange(N)
    MAX_GREEDY = 100
    HARD_CAP = 200

    value = s.copy()
    cost = np.zeros(N, np.float32)
    prev_bidders = np.zeros(N, np.int32)
    prev_have = np.zeros(N, bool)
    it = 0
    top_index = None
    while it < HARD_CAP:
        order = np.argsort(-value, axis=1, kind="stable")
        top_index = order[:, : jpw + 1]
        top_values = np.take_along_axis(value, top_index, axis=1)
        bid_incr = top_values[:, :jpw] - top_values[:, jpw:] + eps
        bids = np.zeros_like(s)
        bids[rows, top_index[:, :jpw]] = bid_incr
        bids[prev_bidders, jobs_idx] = np.where(
            prev_have, eps, bids[prev_bidders, jobs_idx]
        )
        high_bids = bids.max(axis=0)
        high_bidders = bids.argmax(axis=0).astype(np.int32)
        have_bids = high_bids > 0
        done = bool(np.all(have_bids))
        cost = (cost + high_bids).astype(np.float32)
        value = (s - cost).astype(np.float32)
        if it < MAX_GREEDY:
            upd = np.full(N, np.inf, np.float32)
        else:
            upd = s[high_bidders, jobs_idx]
        value[high_bidders, jobs_idx] = np.where(
            have_bids, upd, value[high_bidders, jobs_idx]
        )
        prev_bidders = high_bidders
        prev_have = have_bids
        it += 1
        if done:
            break
    return top_index[:, :jpw].astype(np.int32)


def kernel(input_features, expert_centroids):
    x_flat = np.ascontiguousarray(
        input_features.reshape(-1, input_features.shape[-1])
    ).astype(np.float32, copy=False)
    afft = _device_affinities_T(x_flat, expert_centroids)  # [E, N]
    top_idx = _balanced_assignment_host(afft)
    top_value = np.take_along_axis(afft, top_idx, axis=1).astype(np.float32)
    return top_idx, top_value
